# revision 1
# baseline (speedup 1.0000x reference)
"""Trainium2 Bass kernel for per-batch adaptive 3D histogram binning + linear classifier.

reference semantics (per batch b):
    mins/maxs over N points per dim; scale = 8/rng
    idx = clip(floor((x-min)*scale), 0, 7) per dim
    flat = (idx0*8 + idx1)*8 + idx2  in [0, 512)
    counts = bincount(flat)/N ; logits = counts @ W.T + bias

Strategy (per core, data-parallel over batch across 8 cores):
    - 8 batches/core, points laid out [125 partitions x 800 cols]
    - binning: ACT relu(scale*x+bias) with scale shrunk by (1-1e-6) so the
      x==max point truncates to bin 7 without explicit clips
    - histogram: factored one-hots (hi in [0,32) on DVE at 2x bf16,
      lo in [0,16) on GPSIMD), joint counts via PSUM-accumulated matmuls over
      8-column groups with a block-diagonal trick; both matmul operands are
      single-stride contiguous views
    - logits: counts (fp32 exact) @ (W/N) via a q=4-way folded matmul + bias
"""

import os
import numpy as np
from contextlib import ExitStack

B_FULL = 64
N = 100000
CLASSES = 40
RES = 8
NBINS = RES**3  # 512
NCORES = 8
B_LOC = B_FULL // NCORES  # 8

P = 125            # SBUF partitions used for point data (125*800 = 100000)
COLS = N // P      # 800
HALF = COLS // 4   # 200  (one-hot chunk, for SBUF pressure)
GRP = 8            # columns per matmul group; psum partitions = GRP*LO = 128
NG = HALF // GRP   # 25 groups per chunk
HI = 32
LO = 16

SCALE_EPS = 1e-6   # shrink scale so t(max) < 8 strictly (clip-free)

_CACHE = {}


def _iota_tables():
    import ml_dtypes
    il = np.arange(LO, dtype=np.float32)                       # [LO]
    ih = np.repeat(np.arange(HI, dtype=np.float32), GRP).reshape(HI, GRP)
    return (il.astype(ml_dtypes.bfloat16), ih.astype(ml_dtypes.bfloat16))


def _build_program():
    import concourse.bass as bass
    import concourse.bacc as bacc
    import concourse.tile as tile
    from concourse import mybir
    from concourse.masks import make_identity

    f32 = mybir.dt.float32
    i32 = mybir.dt.int32
    bf16 = mybir.dt.bfloat16
    Alu = mybir.AluOpType
    ActFn = mybir.ActivationFunctionType

    nc = bacc.Bacc(
        "TRN2",
        target_bir_lowering=False,
        debug=False,
        enable_asserts=False,
        num_devices=NCORES,
    )
    x_d = nc.dram_tensor("x", [B_LOC, N, 3], f32, kind="ExternalInput")
    w_d = nc.dram_tensor("W", [CLASSES, NBINS], f32, kind="ExternalInput")
    b_d = nc.dram_tensor("b", [CLASSES], f32, kind="ExternalInput")
    il_d = nc.dram_tensor("iota_lo", [LO], bf16, kind="ExternalInput")
    ih_d = nc.dram_tensor("iota_hi", [HI, GRP], bf16, kind="ExternalInput")
    o_d = nc.dram_tensor("out", [B_LOC, CLASSES], f32, kind="ExternalOutput")
    dbg = os.environ.get("KBG_DEBUG") == "1"
    if dbg:
        dbg_scb = nc.dram_tensor("dbg_scb", [P, 48], f32, kind="ExternalOutput")
        dbg_flat = nc.dram_tensor("dbg_flat", [P, COLS], f32, kind="ExternalOutput")
        dbg_hilo = nc.dram_tensor("dbg_hilo", [P, 2, COLS], f32, kind="ExternalOutput")
        dbg_cnt = nc.dram_tensor("dbg_cnt", [LO, B_LOC, HI], f32, kind="ExternalOutput")
        dbg_ohlo = nc.dram_tensor("dbg_ohlo", [P, NG, GRP, LO], f32, kind="ExternalOutput")
        dbg_ohhi = nc.dram_tensor("dbg_ohhi", [P, NG, HI, GRP], f32, kind="ExternalOutput")

    def reap(ap, dims, extra_offset=0):
        return bass.AP(tensor=ap.tensor, offset=ap.offset + extra_offset, ap=dims)

    with tile.TileContext(nc) as tc, ExitStack() as ctx:
        consts = ctx.enter_context(tc.tile_pool(name="consts", bufs=1))
        xpool = ctx.enter_context(tc.tile_pool(name="xp", bufs=2))
        work = ctx.enter_context(tc.tile_pool(name="work", bufs=1))
        ohpool = ctx.enter_context(tc.tile_pool(name="oh", bufs=2))
        accum = ctx.enter_context(tc.tile_pool(name="acc", bufs=1))
        psum = ctx.enter_context(tc.tile_pool(name="ps", bufs=6, space="PSUM"))
        psum1 = ctx.enter_context(tc.tile_pool(name="ps1", bufs=2, space="PSUM"))

        x_ap = x_d.ap()
        w_ap = w_d.ap()
        b_ap = b_d.ap()
        o_ap = o_d.ap()

        # ------------- constants -------------------------------------------
        iota_lo = consts.tile([P, LO], bf16)     # value l at col l
        nc.sync.dma_start(out=iota_lo[:], in_=reap(il_d.ap(), [[0, P], [1, LO]]))
        iota_hi = consts.tile([P, HI, GRP], bf16)  # value h at (h, t)
        nc.sync.dma_start(out=iota_hi[:],
                          in_=reap(ih_d.ap(), [[0, P], [1, HI * GRP]]))
        ident = consts.tile([128, 128], f32)
        make_identity(nc, ident[:])
        ones_r = consts.tile([1, P], f32)
        nc.vector.memset(ones_r[:], 1.0)

        # ------------- pass 1: per-(batch, dim) min / max -------------------
        # mmall cols [ib*3+d] = per-partition min, [32+ib*3+d] = max
        # (mx block starts at 32 so the transposed rows are 32-aligned)
        K3 = B_LOC * 3
        mmall = accum.tile([P, 64], f32)
        nc.vector.memset(mmall[:], 0.0)
        for ib in range(B_LOC):
            xt = xpool.tile([P, COLS, 3], f32, tag="xt")
            nc.sync.dma_start(out=xt[:], in_=x_ap[ib].rearrange(
                "(p c) d -> p c d", p=P))
            xt_dc = xt[:].rearrange("p c d -> p d c")
            nc.vector.tensor_reduce(out=mmall[:, ib * 3:ib * 3 + 3], in_=xt_dc,
                                    axis=mybir.AxisListType.X, op=Alu.min)
            nc.vector.tensor_reduce(out=mmall[:, 32 + ib * 3:32 + ib * 3 + 3],
                                    in_=xt_dc,
                                    axis=mybir.AxisListType.X, op=Alu.max)

        # transpose [P, 64] -> [64, P], reduce across points -> [64, 1]
        tp1 = psum1.tile([64, 128], f32, tag="aux")
        nc.tensor.transpose(out=tp1[:, :P], in_=mmall[:],
                            identity=ident[:P, :P])
        mnmx = work.tile([64, 1], f32, tag="mnmx")
        nc.vector.memset(mnmx[:], 0.0)
        nc.vector.tensor_reduce(out=mnmx[:K3], in_=tp1[:K3, :P],
                                axis=mybir.AxisListType.X, op=Alu.min)
        nc.vector.tensor_reduce(out=mnmx[32:32 + K3], in_=tp1[32:32 + K3, :P],
                                axis=mybir.AxisListType.X, op=Alu.max)
        # transpose [64, 1] -> [1, 64] row
        tp2 = psum1.tile([1, 64], f32, tag="aux")
        nc.tensor.transpose(out=tp2[:], in_=mnmx[:], identity=ident[:64, :64])
        tp2sb = work.tile([1, 64], f32, tag="tp2sb")
        nc.scalar.copy(out=tp2sb[:], in_=tp2[:])
        # scb row: [1, 0:24] = scale, [1, 24:48] = bias = -min*scale
        rng_r = work.tile([1, K3], f32, tag="rng_r")
        nc.vector.tensor_tensor(out=rng_r[:], in0=tp2sb[:, 32:32 + K3],
                                in1=tp2sb[:, :K3], op=Alu.subtract)
        rcp_r = work.tile([1, K3], f32, tag="rcp_r")
        nc.vector.reciprocal(out=rcp_r[:], in_=rng_r[:])
        scb = work.tile([1, 2 * K3], f32, tag="scb")
        nc.vector.tensor_scalar(out=scb[:, :K3], in0=rcp_r[:],
                                scalar1=float(RES) * (1.0 - SCALE_EPS),
                                scalar2=None, op0=Alu.mult)
        nc.vector.scalar_tensor_tensor(out=scb[:, K3:], in0=tp2sb[:, :K3],
                                       scalar=-1.0, in1=scb[:, :K3],
                                       op0=Alu.mult, op1=Alu.mult)
        # HW f32->i32 convert is round-half-even; shift by -0.5 so the
        # convert computes floor(scale*x + bias_true)
        nc.vector.tensor_scalar(out=scb[:, K3:], in0=scb[:, K3:],
                                scalar1=-0.5, scalar2=None, op0=Alu.add)
        # broadcast to all P partitions: [P, 48] = ones[1,P].T @ scb[1,48]
        tp3 = psum1.tile([P, 2 * K3], f32, tag="aux")
        nc.tensor.matmul(out=tp3[:], lhsT=ones_r[:], rhs=scb[:],
                         start=True, stop=True)
        scbb = accum.tile([P, 2 * K3], f32)
        nc.scalar.copy(out=scbb[:], in_=tp3[:])
        if dbg:
            nc.sync.dma_start(out=dbg_scb.ap(), in_=scbb[:])

        # ------------- pass 2: binning + histogram --------------------------
        counts_all = accum.tile([LO, B_LOC, HI], f32)
        for ib in range(B_LOC):
            xt = xpool.tile([P, COLS, 3], f32, tag="xt")
            nc.sync.dma_start(out=xt[:], in_=x_ap[ib].rearrange(
                "(p c) d -> p c d", p=P))

            # u_d = relu(scale*x_d + bias)  (planar [P, 3, COLS] output)
            u = work.tile([P, 3, COLS], f32, tag="u")
            for d in range(3):
                nc.scalar.activation(
                    out=u[:, d, :], in_=xt[:, :, d], func=ActFn.Relu,
                    bias=scbb[:, K3 + ib * 3 + d:K3 + ib * 3 + d + 1],
                    scale=scbb[:, ib * 3 + d:ib * 3 + d + 1])

            # trunc to int (floor for >=0), back to f32 (exact small ints)
            ui = work.tile([P, 3, COLS], i32, tag="ui")
            nc.vector.tensor_copy(out=ui[:], in_=u[:])
            uf = work.tile([P, 3, COLS], f32, tag="uf")
            nc.vector.tensor_copy(out=uf[:], in_=ui[:])

            # flat = (u0*8 + u1)*8 + u2 ; hi = flat>>4 ; lo = flat&15
            st = work.tile([P, COLS], f32, tag="st")
            nc.vector.scalar_tensor_tensor(out=st[:], in0=uf[:, 0, :],
                                           scalar=8.0, in1=uf[:, 1, :],
                                           op0=Alu.mult, op1=Alu.add)
            flat = work.tile([P, COLS], f32, tag="flat")
            nc.vector.scalar_tensor_tensor(out=flat[:], in0=st[:], scalar=8.0,
                                           in1=uf[:, 2, :], op0=Alu.mult,
                                           op1=Alu.add)
            hi_q = work.tile([P, COLS], f32, tag="hi_q")
            nc.vector.tensor_scalar(out=hi_q[:], in0=flat[:], scalar1=1.0 / 16.0,
                                    scalar2=-0.499, op0=Alu.mult, op1=Alu.add)
            hi_i = work.tile([P, COLS], i32, tag="hi_i")
            nc.vector.tensor_copy(out=hi_i[:], in_=hi_q[:])  # trunc, exact
            hi_b = work.tile([P, COLS], bf16, tag="hi_b")
            nc.scalar.copy(out=hi_b[:], in_=hi_i[:])
            hi_f = work.tile([P, COLS], f32, tag="hi_f")
            nc.vector.tensor_copy(out=hi_f[:], in_=hi_i[:])
            lo_f = work.tile([P, COLS], f32, tag="lo_f")
            nc.vector.scalar_tensor_tensor(out=lo_f[:], in0=hi_f[:],
                                           scalar=-16.0, in1=flat[:],
                                           op0=Alu.mult, op1=Alu.add)
            lo_b = work.tile([P, COLS], bf16, tag="lo_b")
            nc.scalar.copy(out=lo_b[:], in_=lo_f[:])

            if dbg and ib == 0:
                nc.sync.dma_start(out=dbg_flat.ap(), in_=flat[:])
                hb2f = work.tile([P, COLS], f32, tag="hb2f")
                nc.vector.tensor_copy(out=hb2f[:], in_=hi_b[:])
                nc.sync.dma_start(out=dbg_hilo.ap()[:, 0, :], in_=hb2f[:])
                lb2f = work.tile([P, COLS], f32, tag="lb2f")
                nc.vector.tensor_copy(out=lb2f[:], in_=lo_b[:])
                nc.sync.dma_start(out=dbg_hilo.ap()[:, 1, :], in_=lb2f[:])

            # one-hots + matmul-accumulated joint histogram
            # oh_lo layout [g, t, lo] (lo inner)  -> lhsT m = t*LO+lo
            # oh_hi layout [g, hi, t] (t inner)   -> rhs  f = hi*GRP+t
            pt = psum.tile([GRP * LO, GRP * HI], f32, tag="pt")
            nchunk = COLS // HALF
            for h in range(nchunk):
                oh_lo = ohpool.tile([P, NG, GRP, LO], bf16, tag="oh_lo")
                lo_sl = lo_b[:, h * HALF:(h + 1) * HALF]
                in0 = reap(lo_sl, [lo_sl.ap[0], [GRP, NG], [1, GRP], [0, LO]])
                in1 = reap(iota_lo[:],
                           [iota_lo[:].ap[0], [0, NG], [0, GRP], [1, LO]])
                nc.vector.tensor_tensor(out=oh_lo[:], in0=in0, in1=in1,
                                        op=Alu.is_equal)

                oh_hi = ohpool.tile([P, NG, HI, GRP], bf16, tag="oh_hi")
                hi_sl = hi_b[:, h * HALF:(h + 1) * HALF]
                in0h = reap(hi_sl, [hi_sl.ap[0], [GRP, NG], [0, HI], [1, GRP]])
                in1h = reap(iota_hi[:],
                            [iota_hi[:].ap[0], [0, NG], [GRP, HI], [1, GRP]])
                nc.vector.tensor_tensor(out=oh_hi[:], in0=in0h, in1=in1h,
                                        op=Alu.is_equal)

                if dbg and ib == 0 and h == 0:
                    olo2f = work.tile([P, NG, GRP, LO], f32, tag="olo2f")
                    nc.vector.tensor_copy(out=olo2f[:], in_=oh_lo[:])
                    nc.sync.dma_start(out=dbg_ohlo.ap(), in_=olo2f[:])
                    ohi2f = work.tile([P, NG, HI, GRP], f32, tag="ohi2f")
                    nc.vector.tensor_copy(out=ohi2f[:], in_=oh_hi[:])
                    nc.sync.dma_start(out=dbg_ohhi.ap(), in_=ohi2f[:])
                for g in range(NG):
                    lhsT = reap(oh_lo[:], [oh_lo[:].ap[0], [1, GRP * LO]],
                                extra_offset=g * GRP * LO)
                    rhs = reap(oh_hi[:], [oh_hi[:].ap[0], [1, GRP * HI]],
                               extra_offset=g * GRP * HI)
                    nc.tensor.matmul(out=pt[:], lhsT=lhsT, rhs=rhs,
                                     start=(h == 0 and g == 0),
                                     stop=(h == nchunk - 1 and g == NG - 1))

            # copy PSUM->SBUF, gather the 8 diagonal [LO, HI] blocks
            # (partitions [t*LO,(t+1)*LO), free stride GRP offset t), reduce
            ptsb = work.tile([GRP * LO, GRP * HI], f32, tag="ptsb")
            nc.scalar.copy(out=ptsb[:], in_=pt[:])
            diag = work.tile([LO, GRP, HI], f32, tag="diag")
            for t in range(GRP):
                sl = ptsb[t * LO:(t + 1) * LO, :]
                nc.sync.dma_start(out=diag[:, t, :],
                                  in_=reap(sl, [sl.ap[0], [GRP, HI]],
                                           extra_offset=t))
            nc.vector.tensor_reduce(out=counts_all[:, ib, :],
                                    in_=diag[:].rearrange("l t h -> l h t"),
                                    axis=mybir.AxisListType.X, op=Alu.add)

        if dbg:
            nc.sync.dma_start(out=dbg_cnt.ap(), in_=counts_all[:])

        # ------------- final: logits = counts/N @ W.T + b -------------------
        # cnt128[pp, q, ib] = counts(lo=pp%16, hi=q*8+pp//16) of batch ib
        # (flat = q*128 + pp), Wr[pp, q, c] = W[c, q*128+pp] / N
        cnt128 = accum.tile([GRP * LO, 4, B_LOC], f32)
        for j in range(8):
            for q in range(4):
                src = reap(counts_all[:],
                           [counts_all[:].ap[0], [HI, B_LOC]],
                           extra_offset=j + q * 8)
                nc.sync.dma_start(out=cnt128[j * 16:(j + 1) * 16, q, :],
                                  in_=src)

        wr = accum.tile([GRP * LO, 4, CLASSES], f32)
        for q in range(4):
            w_src = reap(w_ap, [[1, 128], [NBINS, CLASSES]],
                         extra_offset=q * 128)
            nc.sync.dma_start(out=wr[:, q, :], in_=w_src)
        wrs = accum.tile([GRP * LO, 4, CLASSES], f32)
        nc.vector.tensor_scalar(out=wrs[:], in0=wr[:], scalar1=1.0 / N,
                                scalar2=None, op0=Alu.mult)

        pf = psum1.tile([4 * B_LOC, 4 * CLASSES], f32, tag="aux")
        nc.tensor.matmul(out=pf[:], lhsT=cnt128[:].rearrange("p q b -> p (q b)"),
                         rhs=wrs[:].rearrange("p q c -> p (q c)"),
                         start=True, stop=True)

        pfsb = work.tile([4 * B_LOC, 4 * CLASSES], f32, tag="pfsb")
        nc.scalar.copy(out=pfsb[:], in_=pf[:])
        fin = work.tile([B_LOC, 4, CLASSES], f32, tag="fin")
        for q in range(4):
            nc.sync.dma_start(out=fin[:, q, :],
                              in_=pfsb[q * B_LOC:(q + 1) * B_LOC,
                                       q * CLASSES:(q + 1) * CLASSES])
        biast = work.tile([B_LOC, CLASSES], f32, tag="biast")
        nc.sync.dma_start(out=biast[:],
                          in_=reap(b_ap, [[0, B_LOC], [1, CLASSES]]))
        red = work.tile([B_LOC, CLASSES], f32, tag="red")
        nc.vector.tensor_reduce(out=red[:],
                                in_=fin[:].rearrange("b q c -> b c q"),
                                axis=mybir.AxisListType.X, op=Alu.add)
        logits = work.tile([B_LOC, CLASSES], f32, tag="logits")
        nc.vector.tensor_tensor(out=logits[:], in0=red[:], in1=biast[:],
                                op=Alu.add)
        nc.sync.dma_start(out=o_ap, in_=logits[:])

    nc.compile()
    return nc


def _get_program():
    if "nc" not in _CACHE:
        _CACHE["nc"] = _build_program()
    return _CACHE["nc"]


def _in_maps(x, W, b):
    il, ih = _iota_tables()
    return [
        {
            "x": np.ascontiguousarray(x[i * B_LOC:(i + 1) * B_LOC]),
            "W": np.ascontiguousarray(W),
            "b": np.ascontiguousarray(b),
            "iota_lo": il,
            "iota_hi": ih,
        }
        for i in range(NCORES)
    ]


def _run(x, W, b, trace=False, trace_cores=None):
    from concourse.bass_utils import run_bass_kernel_spmd

    nc = _get_program()
    return run_bass_kernel_spmd(nc, _in_maps(x, W, b),
                                core_ids=list(range(NCORES)), trace=trace,
                                trace_cores=trace_cores)


def kernel(**inputs):
    x = inputs["x"]
    W = inputs["W"]
    b = inputs["b"]
    assert x.shape == (B_FULL, N, 3) and x.dtype == np.float32
    res = _run(x, W, b)
    return np.concatenate([r["out"] for r in res.results], axis=0)

